# revision 3
# baseline (speedup 1.0000x reference)
"""Trainium2 Bass kernel for feature_smoothing: trace(X^T L_norm X).

v2: host symmetrizes A = (adj + adj^T)/2 before sharding (per the
sharding hint, which already frames the problem as sharding "adj and L").
With A symmetric, deg = colsum(A), and core c's column block A[:, block_c]
contains ALL rows of those columns: deg[block_c] is computed entirely on
core c by PE ones-matmuls (partition contraction) -- no rowsum on the
vector engines (was ~60us of ACT/DVE work in v1) and no 36KB partials
collective.

Pipeline: the column block streams in NCH column chunks.  Chunk k's
colsum -> deg -> 1KB AllGather -> rinv -> Y rows unlock while chunk k+1
is still DMAing, so the phase-C matmul P = A[:, block]^T @ Y overlaps
the adj stream instead of serializing behind a full-matrix barrier.

loss = sum_i w_i ||X_i||^2 - sum_i r_i (X_i . P_i),  w = deg/(deg+eps),
r = (deg+eps)^-1/2, Y = YS * r * X in fp8.

Host prep: symmetrize + fp8 cast + pack every DMA source partition-
contiguous (elem >= 1KB -- no <512B descriptor penalty).  Host post:
O(N) dots (gather/unshard glue), as baseline.

PSUM plan: 8 phase-C accumulators [128,512]f32 fill all 8 banks; the 4
colsum accumulators time-share via pool rotation with the odd-block
accumulators (cs_k's buffer is reused by mm[2k+1], whose first write
can't precede chunk k's arrival anyway).
"""

import sys

if "/opt/trn_rl_repo" not in sys.path:
    sys.path.insert(0, "/opt/trn_rl_repo")

import numpy as np

N = 8192
F = 512
M = 8            # cores
C = N // M       # columns per core = 1024
T = N // 128     # 128-row tiles of the full dim = 64
TC = C // 128    # 128-col tiles of the local block = 8
EPS = 1e-5
YS = 64.0        # fp8 scale for Y

# column chunks of the local block, in 128-col units (even widths only:
# DR pairing).  Two chunks: the collective cost model has a ~15us
# constant per AllGather and serializes them, so fewer, earlier gathers
# beat a finer pipeline.
CHUNKS = [4, 4]
NCH = len(CHUNKS)
COFF = [sum(CHUNKS[:k]) for k in range(NCH)]   # 128-col offsets

_CACHE = {}


def _build_bass(n_devices=M, use_collectives=True):
    import concourse.mybir as mybir
    import concourse.tile as tile
    from concourse import bacc

    f32 = mybir.dt.float32
    bf16 = mybir.dt.bfloat16
    fp8 = mybir.dt.float8e4
    ALU = mybir.AluOpType
    ACTF = mybir.ActivationFunctionType
    DR = mybir.MatmulPerfMode.DoubleRow

    nc = bacc.Bacc("TRN2", target_bir_lowering=False, debug=False,
                   num_devices=n_devices)

    # prepacked [128, ...] partition-contiguous sources (host does layout)
    adjb = nc.dram_tensor("adjb", [128, T * C], fp8, kind="ExternalInput").ap()
    xb = nc.dram_tensor("xb", [128, T * F], fp8, kind="ExternalInput").ap()
    xlb = nc.dram_tensor("xlb", [128, TC * F], fp8, kind="ExternalInput").ap()
    out_h = nc.dram_tensor("out_h", [128, T], f32, kind="ExternalOutput").ap()
    out_xsq = nc.dram_tensor("out_xsq", [128, TC], f32, kind="ExternalOutput").ap()
    out_q = nc.dram_tensor("out_q", [128, TC], f32, kind="ExternalOutput").ap()

    with tile.TileContext(nc) as tc:
        with (
            tc.tile_pool(name="big", bufs=1) as big,
            tc.tile_pool(name="vec", bufs=1) as vec,
            tc.tile_pool(name="ps", bufs=8, space="PSUM") as ps,
            tc.tile_pool(name="dram", bufs=1, space="DRAM") as dram,
        ):
            ones2 = vec.tile([128, 2, 1], fp8)
            nc.vector.memset(ones2[:], 1.0)

            # resident data -- per-chunk/per-wave tiles so the dependency
            # tracker never sees false WARs between stream pieces and the
            # previous chunk's readers
            a8c = [big.tile([128, T * CHUNKS[k] * 128], fp8, name=f"a8c{k}")
                   for k in range(NCH)]
            xw = [big.tile([128, M * CHUNKS[k], F], fp8, name=f"xw{k}")
                  for k in range(NCH)]
            y = big.tile([128, T, F], fp8, name="y")
            xl = big.tile([128, TC, F], fp8, name="xl")

            # small maps; [128, T] tensors use global tile layout:
            # column TC*a + t <-> global row a*1024 + t*128 + p
            degm = vec.tile([128, T], f32, name="degm")
            rec = vec.tile([128, T], f32, name="rec")
            rec2 = vec.tile([128, T], f32, name="rec2")
            rinv64 = vec.tile([128, T], f32, name="rinv64")
            xsq = vec.tile([128, TC], f32, name="xsq")
            q8 = vec.tile([128, TC], f32, name="q8")
            cs_sb = vec.tile([128, TC], f32, name="cs_sb")
            dumpf = vec.tile([128, F], bf16, name="dumpf")    # ACT square sink
            dumpfv = vec.tile([128, F], bf16, name="dumpfv")  # DVE square sink

            ag_in = [dram.tile([128, CHUNKS[k]], f32, name=f"agin{k}")
                     for k in range(NCH)]
            ag_out = [dram.tile([n_devices * 128, CHUNKS[k]], f32,
                                 name=f"agout{k}")
                      for k in range(NCH)]

            # PSUM: cs accumulators first (bufs 0..3), then even mm blocks
            # (fresh bufs 4..7), then odd mm blocks (reuse cs bufs: mm[2k+1]
            # takes cs_k's buffer, safe since both gate on chunk k's DMA).
            cs_ps = [ps.tile([128, CHUNKS[k]], f32, tag="ps", name=f"cs{k}")
                     for k in range(NCH)]
            mm = [None] * TC
            # rotation: cs0->b0, cs1->b1, mm0..5 fresh, mm6/7 reuse cs bufs
            # (mm6/7 are chunk-1 blocks whose first write gates on chunk 1
            # anyway)
            for b in [0, 1, 2, 3, 4, 5, 6, 7]:
                mm[b] = ps.tile([128, F], f32, tag="ps", name=f"mm{b}")

            # chunk-k view: [128, T, w_k*128]
            def a_view(k):
                w = CHUNKS[k] * 128
                return a8c[k][:].rearrange("p (t c) -> p t c", c=w)

            def block_view(t2d):
                return t2d.rearrange("p (a t) -> p a t", a=M)

            # ---------------- DMA issue (SP queue, FIFO) ------------------
            # Big transfers split into ~0.6us pieces so the transfer rate
            # matches the HWDGE config rate: the DMA-engine queue stays
            # ~empty and chain DMAs slot in with sub-us latency, with no
            # SP-side throttling needed.
            ASUB = 16
            XSUB = 8

            def emit_dma_a8(k):
                base = T * COFF[k] * 128
                tot = T * CHUNKS[k] * 128
                step = tot // ASUB
                for s in range(ASUB):
                    nc.sync.dma_start(
                        a8c[k][:, s * step:(s + 1) * step],
                        adjb[:, base + s * step:base + (s + 1) * step])

            def emit_dma_x(k):
                w = CHUNKS[k]
                xa = xw[k][:].rearrange("p (a j) c -> p a j c", j=w)
                xba = xb[:].rearrange("p (a t c) -> p a t c", a=M, t=TC)
                astep = M // XSUB
                for s in range(XSUB):
                    nc.sync.dma_start(
                        xa[:, s * astep:(s + 1) * astep, :, :],
                        xba[:, s * astep:(s + 1) * astep,
                            COFF[k]:COFF[k] + CHUNKS[k], :])

            # ------------- per-chunk: colsum -> gather -> rinv -------------
            def emit_colsum(k):
                av = a_view(k)
                NP = T // 2
                for m in range(CHUNKS[k]):
                    # full colsum over all 64 row-tiles...
                    for t2 in range(NP):
                        nc.tensor.matmul(
                            cs_ps[k][:, m:m + 1],
                            av[:, 2 * t2:2 * t2 + 2, m * 128:(m + 1) * 128],
                            ones2[:],
                            start=(t2 == 0), stop=False,
                            perf_mode=DR)
                    # ...plus the wave-k rows once more: they were halved on
                    # host (chunk-space diagonal), so the resum restores the
                    # exact colsum in the same accumulator
                    NPW = CHUNKS[k] // 2
                    for a in range(M):
                        for p in range(NPW):
                            t0 = TC * a + COFF[k] + 2 * p
                            nc.tensor.matmul(
                                cs_ps[k][:, m:m + 1],
                                av[:, t0:t0 + 2, m * 128:(m + 1) * 128],
                                ones2[:],
                                start=False,
                                stop=(a == M - 1 and p == NPW - 1),
                                perf_mode=DR)
                co = COFF[k]
                w = CHUNKS[k]
                nc.vector.tensor_copy(cs_sb[:, co:co + w], cs_ps[k][:])

            def emit_ag_write(k):
                co, w = COFF[k], CHUNKS[k]
                # SP FIFO: the wait on the colsum drains the DMA queue, so
                # this write (and the collective behind it) start immediately
                nc.sync.dma_start(ag_in[k][:], cs_sb[:, co:co + w])

            def emit_collective(k):
                co, w = COFF[k], CHUNKS[k]
                if use_collectives:
                    grp = [list(range(n_devices))]
                    nc.gpsimd.collective_compute(
                        "AllGather", ALU.bypass, replica_groups=grp,
                        ins=[ag_in[k][:]], outs=[ag_out[k][:]])
                else:
                    nc.gpsimd.dma_start(ag_out[k][0:128, :], ag_in[k][:])
            def emit_rg(k):
                co, w = COFF[k], CHUNKS[k]
                # gathered deg chunk -> degm on ACT HWDGE; emitted at its
                # availability point so it never head-blocks ACT compute
                if n_devices == M:
                    nc.scalar.dma_start(
                        block_view(degm[:])[:, :, co:co + w],
                        ag_out[k][:].rearrange("(a p) f -> p a f", p=128))
                else:
                    # single-core sim: structural stand-in (timing only)
                    nc.scalar.dma_start(
                        block_view(degm[:])[:, 0:1, co:co + w],
                        ag_out[k][0:128, :].rearrange("(a p) f -> p a f",
                                                      p=128))

            def emit_rinv(k):
                co, w = COFF[k], CHUNKS[k]
                dv = block_view(degm[:])[:, :, co:co + w]
                rv = block_view(rec[:])[:, :, co:co + w]
                r2 = block_view(rec2[:])[:, :, co:co + w]
                iv = block_view(rinv64[:])[:, :, co:co + w]
                # rec2 = 1/(deg+eps); rinv64 = sqrt(YS^2 * rec2)
                nc.vector.tensor_scalar(rv, dv, EPS, 0.0,
                                        op0=ALU.add, op1=ALU.add)
                nc.vector.reciprocal(r2, rv)
                nc.scalar.activation(iv, r2, ACTF.Sqrt, scale=YS * YS)

            def emit_y(k, engines):
                co = COFF[k]
                i = 0
                for a in range(M):
                    for j in range(CHUNKS[k]):
                        t = TC * a + co + j
                        xs = xw[k][:, a * CHUNKS[k] + j, :]
                        e = engines[i % len(engines)]
                        i += 1
                        if e == "act":
                            nc.scalar.mul(y[:, t, :], xs,
                                          rinv64[:, t:t + 1])
                        elif e == "pool":
                            nc.gpsimd.tensor_scalar_mul(y[:, t, :], xs,
                                                        rinv64[:, t:t + 1])
                        else:
                            nc.vector.tensor_scalar_mul(y[:, t, :], xs,
                                                        rinv64[:, t:t + 1])

            # term1 needs ||X_i||^2 only for LOCAL rows (host gathers all
            # cores' maps): 8 tiles from xl instead of 64 from x
            def emit_xsq_local(engines):
                for b in range(TC):
                    e = engines[b % len(engines)]
                    if e == "act":
                        nc.scalar.activation(dumpf[:], xl[:, b, :],
                                             ACTF.Square,
                                             accum_out=xsq[:, b:b + 1])
                    else:
                        nc.vector.scalar_tensor_tensor(
                            dumpfv[:], xl[:, b, :], 1.0, xl[:, b, :],
                            op0=ALU.mult, op1=ALU.mult,
                            accum_out=xsq[:, b:b + 1])

            # phase-C cell (row-wave w, col-chunk k), only emitted for
            # w <= k: by symmetry of A, S_wk == S_kw, so the lower triangle
            # is folded in by double-counting the off-diagonal accumulation
            # (qoff drain below).  Halves the matmul work and leaves only
            # the small diagonal cell (k,k) gated on gather k.
            def emit_cell(w, k, start, stop, drain=False):
                assert CHUNKS[w] % 2 == 0, "DR pairing needs even waves"
                NPW = CHUNKS[w] // 2
                av = a_view(k)
                # m-major: block b fully accumulates before b+1 starts, so
                # its drain overlaps the next block's matmuls
                for m in range(CHUNKS[k]):
                    b = COFF[k] + m
                    for a in range(M):
                        for p in range(NPW):
                            t0 = TC * a + COFF[w] + 2 * p
                            nc.tensor.matmul(
                                mm[b][:],
                                av[:, t0:t0 + 2, m * 128:(m + 1) * 128],
                                y[:, t0:t0 + 2, :],
                                start=(start and a == 0 and p == 0),
                                stop=(stop and a == M - 1 and p == NPW - 1),
                                perf_mode=DR)
                    if drain:
                        emit_drain(b)

            # q_b = X_lb . P_b; P_b holds offdiag + 0.5*diag, so
            # term2 = 2 * sum r * q (host side)
            def emit_drain(b):
                nc.vector.scalar_tensor_tensor(
                    xl[:, b, :], mm[b][:], 1.0, xl[:, b, :],
                    op0=ALU.mult, op1=ALU.mult, accum_out=q8[:, b:b + 1])

            # ----------------- emission schedule --------------------------
            # Both collectives issue back-to-back as early as possible (they
            # serialize on the collective cores and dominate the critical
            # path); X waves stream after a8 since Y can't build before the
            # gathers return anyway.
            YENG = ["dve", "act", "dve", "act", "dve"]

            emit_dma_a8(0)
            emit_colsum(0)
            emit_ag_write(0)          # SP throttle: queue hole at cs0
            emit_collective(0)

            emit_dma_a8(1)
            emit_colsum(1)
            emit_ag_write(1)          # SP throttle: queue hole at cs1
            emit_collective(1)

            nc.sync.dma_start(
                xl[:], xlb[:].rearrange("p (t c) -> p t c", c=F))
            emit_dma_x(0)
            emit_dma_x(1)

            emit_xsq_local(["dve", "act"])
            emit_rg(0)
            emit_rinv(0)
            emit_y(0, YENG)
            emit_cell(0, 0, start=True, stop=True, drain=True)  # diag (0,0)

            emit_rg(1)
            emit_rinv(1)
            emit_y(1, YENG)
            emit_cell(0, 1, start=True, stop=False)
            nc.sync.dma_start(out_xsq[:], xsq[:])
            nc.sync.dma_start(out_h[:], degm[:])
            emit_cell(1, 1, start=False, stop=True, drain=True)  # (1,1): tail
            nc.sync.dma_start(out_q[:], q8[:])

    nc.compile()
    return nc


def _get_nc():
    if "nc" not in _CACHE:
        _CACHE["nc"] = _build_bass()
    return _CACHE["nc"]


def _pack_tiles(arr, rows_per_tile=128):
    """[n*128, c] -> [128, n*c] partition-contiguous (p-major tiles)."""
    n = arr.shape[0] // rows_per_tile
    c = arr.shape[1]
    return np.ascontiguousarray(
        arr.reshape(n, rows_per_tile, c).transpose(1, 0, 2).reshape(
            rows_per_tile, n * c))


def _host_inputs(adj, X):
    import ml_dtypes
    f8 = ml_dtypes.float8_e4m3

    A = (0.5 * (adj + adj.T)).astype(f8)
    Xb = np.asarray(X, dtype=np.float32).astype(f8)
    xb_p = _pack_tiles(Xb)          # [128, T*F]

    in_maps = []
    for c in range(M):
        blk = A[:, c * C:(c + 1) * C]
        parts = []
        for k in range(NCH):
            o = COFF[k] * 128
            w = CHUNKS[k] * 128
            sub = np.ascontiguousarray(blk[:, o:o + w])
            # halve the wave-k rows (the chunk-space diagonal): the triangle
            # scheme then needs only ONE drain per block (term2 = 2*sum r*q);
            # exact in fp8 (exponent decrement).  The colsum correction
            # (cs_ps2) re-adds the halved partial on device.
            mask = ((np.arange(N) % C) >= o) & ((np.arange(N) % C) < o + w)
            sub[mask] = (sub[mask].astype(np.float32) * 0.5).astype(sub.dtype)
            parts.append(_pack_tiles(sub))
        adjb = np.concatenate(parts, axis=1)
        xlb = _pack_tiles(np.ascontiguousarray(Xb[c * C:(c + 1) * C, :]))
        in_maps.append({"adjb": adjb, "xb": xb_p, "xlb": xlb})
    return in_maps


def kernel(adj: np.ndarray, X: np.ndarray) -> np.ndarray:
    from concourse import bass_utils

    adj = np.asarray(adj, dtype=np.float32)
    X = np.ascontiguousarray(np.asarray(X, dtype=np.float32))
    nc = _get_nc()
    in_maps = _host_inputs(adj, X)

    res = bass_utils.run_bass_kernel_spmd(nc, in_maps, core_ids=list(range(M)))
    results = res.results

    # host-side O(N) reduction (gather/unshard glue)
    deg = results[0]["out_h"].astype(np.float64).T.reshape(-1)
    xsq = np.empty(N, dtype=np.float64)
    for c in range(M):
        xsq[c * C:(c + 1) * C] = results[c]["out_xsq"].astype(
            np.float64).T.reshape(-1)
    w = deg / (deg + EPS)
    rinv = 1.0 / np.sqrt(deg + EPS)
    term1 = float(np.dot(w, xsq))

    q = np.empty(N, dtype=np.float64)
    for c in range(M):
        q[c * C:(c + 1) * C] = results[c]["out_q"].astype(np.float64).T.reshape(-1)
    term2 = 2.0 * float(np.dot(rinv, q)) / YS

    return np.float32(term1 - term2)


if __name__ == "__main__":
    rng = np.random.default_rng(0)
    adj = rng.random((N, N), dtype=np.float32)
    X = rng.standard_normal((N, F), dtype=np.float32)
    print("loss:", kernel(adj, X))


# revision 4
# speedup vs baseline: 1.0111x; 1.0111x over previous
"""Trainium2 Bass kernel for feature_smoothing: trace(X^T L_norm X).

v2: host symmetrizes A = (adj + adj^T)/2 before sharding (per the
sharding hint, which already frames the problem as sharding "adj and L").
With A symmetric, deg = colsum(A), and core c's column block A[:, block_c]
contains ALL rows of those columns: deg[block_c] is computed entirely on
core c by PE ones-matmuls (partition contraction) -- no rowsum on the
vector engines (was ~60us of ACT/DVE work in v1) and no 36KB partials
collective.

Pipeline: the column block streams in NCH column chunks.  Chunk k's
colsum -> deg -> 1KB AllGather -> rinv -> Y rows unlock while chunk k+1
is still DMAing, so the phase-C matmul P = A[:, block]^T @ Y overlaps
the adj stream instead of serializing behind a full-matrix barrier.

loss = sum_i w_i ||X_i||^2 - sum_i r_i (X_i . P_i),  w = deg/(deg+eps),
r = (deg+eps)^-1/2, Y = YS * r * X in fp8.

Host prep: symmetrize + fp8 cast + pack every DMA source partition-
contiguous (elem >= 1KB -- no <512B descriptor penalty).  Host post:
O(N) dots (gather/unshard glue), as baseline.

PSUM plan: 8 phase-C accumulators [128,512]f32 fill all 8 banks; the 4
colsum accumulators time-share via pool rotation with the odd-block
accumulators (cs_k's buffer is reused by mm[2k+1], whose first write
can't precede chunk k's arrival anyway).
"""

import sys

if "/opt/trn_rl_repo" not in sys.path:
    sys.path.insert(0, "/opt/trn_rl_repo")

import numpy as np

N = 8192
F = 512
M = 8            # cores
C = N // M       # columns per core = 1024
T = N // 128     # 128-row tiles of the full dim = 64
TC = C // 128    # 128-col tiles of the local block = 8
EPS = 1e-5
YS = 64.0        # fp8 scale for Y

# column chunks of the local block, in 128-col units (even widths only:
# DR pairing).  Two chunks: the collective cost model has a ~15us
# constant per AllGather and serializes them, so fewer, earlier gathers
# beat a finer pipeline.
CHUNKS = [4, 4]
NCH = len(CHUNKS)
COFF = [sum(CHUNKS[:k]) for k in range(NCH)]   # 128-col offsets

_CACHE = {}


def _build_bass(n_devices=M, use_collectives=True):
    import concourse.mybir as mybir
    import concourse.tile as tile
    from concourse import bacc

    f32 = mybir.dt.float32
    bf16 = mybir.dt.bfloat16
    fp8 = mybir.dt.float8e4
    ALU = mybir.AluOpType
    ACTF = mybir.ActivationFunctionType
    DR = mybir.MatmulPerfMode.DoubleRow

    nc = bacc.Bacc("TRN2", target_bir_lowering=False, debug=False,
                   num_devices=n_devices)

    # prepacked [128, ...] partition-contiguous sources (host does layout)
    adjb = nc.dram_tensor("adjb", [128, T * C], fp8, kind="ExternalInput").ap()
    xb = nc.dram_tensor("xb", [128, T * F], fp8, kind="ExternalInput").ap()
    xlb = nc.dram_tensor("xlb", [128, TC * F], fp8, kind="ExternalInput").ap()
    out_h = nc.dram_tensor("out_h", [128, T], f32, kind="ExternalOutput").ap()
    out_xsq = nc.dram_tensor("out_xsq", [128, TC], f32, kind="ExternalOutput").ap()
    out_q = nc.dram_tensor("out_q", [128, TC], f32, kind="ExternalOutput").ap()

    with tile.TileContext(nc) as tc:
        with (
            tc.tile_pool(name="big", bufs=1) as big,
            tc.tile_pool(name="vec", bufs=1) as vec,
            tc.tile_pool(name="ps", bufs=8, space="PSUM") as ps,
            tc.tile_pool(name="dram", bufs=1, space="DRAM") as dram,
        ):
            ones2 = vec.tile([128, 2, 1], fp8)
            nc.vector.memset(ones2[:], 1.0)

            # resident data -- per-chunk/per-wave tiles so the dependency
            # tracker never sees false WARs between stream pieces and the
            # previous chunk's readers
            a8c = [big.tile([128, T * CHUNKS[k] * 128], fp8, name=f"a8c{k}")
                   for k in range(NCH)]
            xw = [big.tile([128, M * CHUNKS[k], F], fp8, name=f"xw{k}")
                  for k in range(NCH)]
            y = big.tile([128, T, F], fp8, name="y")
            xl = big.tile([128, TC, F], fp8, name="xl")

            # small maps; [128, T] tensors use global tile layout:
            # column TC*a + t <-> global row a*1024 + t*128 + p
            degm = vec.tile([128, T], f32, name="degm")
            rec = vec.tile([128, T], f32, name="rec")
            rec2 = vec.tile([128, T], f32, name="rec2")
            rinv64 = vec.tile([128, T], f32, name="rinv64")
            xsq = vec.tile([128, TC], f32, name="xsq")
            q8 = vec.tile([128, TC], f32, name="q8")
            cs_sb = vec.tile([128, TC], f32, name="cs_sb")
            dumpf = vec.tile([128, F], bf16, name="dumpf")    # ACT square sink
            dumpfv = vec.tile([128, F], bf16, name="dumpfv")  # DVE square sink

            ag_in = [dram.tile([128, CHUNKS[k]], f32, name=f"agin{k}")
                     for k in range(NCH)]
            ag_out = [dram.tile([n_devices * 128, CHUNKS[k]], f32,
                                 name=f"agout{k}")
                      for k in range(NCH)]

            # PSUM: cs accumulators first (bufs 0..3), then even mm blocks
            # (fresh bufs 4..7), then odd mm blocks (reuse cs bufs: mm[2k+1]
            # takes cs_k's buffer, safe since both gate on chunk k's DMA).
            cs_ps = [ps.tile([128, CHUNKS[k]], f32, tag="ps", name=f"cs{k}")
                     for k in range(NCH)]
            mm = [None] * TC
            # rotation: cs0->b0, cs1->b1, mm0..5 fresh, mm6/7 reuse cs bufs
            # (mm6/7 are chunk-1 blocks whose first write gates on chunk 1
            # anyway)
            for b in [0, 1, 2, 3, 4, 5, 6, 7]:
                mm[b] = ps.tile([128, F], f32, tag="ps", name=f"mm{b}")

            # chunk-k view: [128, T, w_k*128]
            def a_view(k):
                w = CHUNKS[k] * 128
                return a8c[k][:].rearrange("p (t c) -> p t c", c=w)

            def block_view(t2d):
                return t2d.rearrange("p (a t) -> p a t", a=M)

            # ---------------- DMA issue (SP queue, FIFO) ------------------
            # Big transfers split into ~0.6us pieces so the transfer rate
            # matches the HWDGE config rate: the DMA-engine queue stays
            # ~empty and chain DMAs slot in with sub-us latency, with no
            # SP-side throttling needed.
            ASUB = 16
            XSUB = 8

            def emit_dma_a8(k):
                base = T * COFF[k] * 128
                tot = T * CHUNKS[k] * 128
                step = tot // ASUB
                for s in range(ASUB):
                    nc.sync.dma_start(
                        a8c[k][:, s * step:(s + 1) * step],
                        adjb[:, base + s * step:base + (s + 1) * step])

            def emit_dma_x(k):
                w = CHUNKS[k]
                xa = xw[k][:].rearrange("p (a j) c -> p a j c", j=w)
                xba = xb[:].rearrange("p (a t c) -> p a t c", a=M, t=TC)
                astep = M // XSUB
                for s in range(XSUB):
                    nc.sync.dma_start(
                        xa[:, s * astep:(s + 1) * astep, :, :],
                        xba[:, s * astep:(s + 1) * astep,
                            COFF[k]:COFF[k] + CHUNKS[k], :])

            # ------------- per-chunk: colsum -> gather -> rinv -------------
            def emit_colsum(k):
                av = a_view(k)
                NP = T // 2
                for m in range(CHUNKS[k]):
                    # full colsum over all 64 row-tiles...
                    for t2 in range(NP):
                        nc.tensor.matmul(
                            cs_ps[k][:, m:m + 1],
                            av[:, 2 * t2:2 * t2 + 2, m * 128:(m + 1) * 128],
                            ones2[:],
                            start=(t2 == 0), stop=False,
                            perf_mode=DR)
                    # ...plus this column strip's width-2 sub-diagonal rows
                    # once more: they were halved on host, so the resum
                    # restores the exact colsum in the same accumulator
                    kp = 2 * k + (m // 2)      # width-2 sub-chunk index
                    for a in range(M):
                        t0 = TC * a + 2 * kp
                        nc.tensor.matmul(
                            cs_ps[k][:, m:m + 1],
                            av[:, t0:t0 + 2, m * 128:(m + 1) * 128],
                            ones2[:],
                            start=False,
                            stop=(a == M - 1),
                            perf_mode=DR)
                co = COFF[k]
                w = CHUNKS[k]
                nc.vector.tensor_copy(cs_sb[:, co:co + w], cs_ps[k][:])

            def emit_ag_write(k):
                co, w = COFF[k], CHUNKS[k]
                # SP FIFO: the wait on the colsum drains the DMA queue, so
                # this write (and the collective behind it) start immediately
                nc.sync.dma_start(ag_in[k][:], cs_sb[:, co:co + w])

            def emit_collective(k):
                co, w = COFF[k], CHUNKS[k]
                if use_collectives:
                    grp = [list(range(n_devices))]
                    nc.gpsimd.collective_compute(
                        "AllGather", ALU.bypass, replica_groups=grp,
                        ins=[ag_in[k][:]], outs=[ag_out[k][:]])
                else:
                    nc.gpsimd.dma_start(ag_out[k][0:128, :], ag_in[k][:])
            def emit_rg(k):
                co, w = COFF[k], CHUNKS[k]
                # gathered deg chunk -> degm on ACT HWDGE; emitted at its
                # availability point so it never head-blocks ACT compute
                if n_devices == M:
                    nc.scalar.dma_start(
                        block_view(degm[:])[:, :, co:co + w],
                        ag_out[k][:].rearrange("(a p) f -> p a f", p=128))
                else:
                    # single-core sim: structural stand-in (timing only)
                    nc.scalar.dma_start(
                        block_view(degm[:])[:, 0:1, co:co + w],
                        ag_out[k][0:128, :].rearrange("(a p) f -> p a f",
                                                      p=128))

            def emit_rinv(k):
                co, w = COFF[k], CHUNKS[k]
                dv = block_view(degm[:])[:, :, co:co + w]
                rv = block_view(rec[:])[:, :, co:co + w]
                r2 = block_view(rec2[:])[:, :, co:co + w]
                iv = block_view(rinv64[:])[:, :, co:co + w]
                # rec2 = 1/(deg+eps); rinv64 = sqrt(YS^2 * rec2)
                nc.vector.tensor_scalar(rv, dv, EPS, 0.0,
                                        op0=ALU.add, op1=ALU.add)
                nc.vector.reciprocal(r2, rv)
                nc.scalar.activation(iv, r2, ACTF.Sqrt, scale=YS * YS)

            def emit_y(k, engines):
                co = COFF[k]
                i = 0
                for a in range(M):
                    for j in range(CHUNKS[k]):
                        t = TC * a + co + j
                        xs = xw[k][:, a * CHUNKS[k] + j, :]
                        e = engines[i % len(engines)]
                        i += 1
                        if e == "act":
                            nc.scalar.mul(y[:, t, :], xs,
                                          rinv64[:, t:t + 1])
                        elif e == "pool":
                            nc.gpsimd.tensor_scalar_mul(y[:, t, :], xs,
                                                        rinv64[:, t:t + 1])
                        else:
                            nc.vector.tensor_scalar_mul(y[:, t, :], xs,
                                                        rinv64[:, t:t + 1])

            # term1 needs ||X_i||^2 only for LOCAL rows (host gathers all
            # cores' maps): 8 tiles from xl instead of 64 from x
            def emit_xsq_local(engines):
                for b in range(TC):
                    e = engines[b % len(engines)]
                    if e == "act":
                        nc.scalar.activation(dumpf[:], xl[:, b, :],
                                             ACTF.Square,
                                             accum_out=xsq[:, b:b + 1])
                    else:
                        nc.vector.scalar_tensor_tensor(
                            dumpfv[:], xl[:, b, :], 1.0, xl[:, b, :],
                            op0=ALU.mult, op1=ALU.mult,
                            accum_out=xsq[:, b:b + 1])

            # phase-C cell (row-wave w, col-chunk k), only emitted for
            # w <= k: by symmetry of A, S_wk == S_kw, so the lower triangle
            # is folded in by double-counting the off-diagonal accumulation
            # (qoff drain below).  Halves the matmul work and leaves only
            # the small diagonal cell (k,k) gated on gather k.
            # cells live at width-2 granularity (wp, kp in 0..T//(2M)-1),
            # finer than the gather chunks: the symmetric triangle then
            # drops more of the lower half (10 of 16 sub-cells) and the
            # tail diag shrinks.  m-major: each block's drain overlaps the
            # next block's matmuls.
            def emit_cell(wp, kp, start, stop, drain=False):
                # gather chunk containing width-2 sub-chunk kp
                k = next(kk for kk in range(NCH)
                         if COFF[kk] <= 2 * kp < COFF[kk] + CHUNKS[kk])
                av = a_view(k)
                coff_cols = kp * 256 - COFF[k] * 128
                for m2 in range(2):
                    b = kp * 2 + m2
                    for a in range(M):
                        t0 = TC * a + 2 * wp
                        nc.tensor.matmul(
                            mm[b][:],
                            av[:, t0:t0 + 2,
                               coff_cols + m2 * 128:coff_cols + (m2 + 1) * 128],
                            y[:, t0:t0 + 2, :],
                            start=(start and a == 0),
                            stop=(stop and a == M - 1),
                            perf_mode=DR)
                    if drain:
                        emit_drain(b)

            # q_b = X_lb . P_b; P_b holds offdiag + 0.5*diag, so
            # term2 = 2 * sum r * q (host side)
            def emit_drain(b):
                nc.vector.scalar_tensor_tensor(
                    xl[:, b, :], mm[b][:], 1.0, xl[:, b, :],
                    op0=ALU.mult, op1=ALU.mult, accum_out=q8[:, b:b + 1])

            # ----------------- emission schedule --------------------------
            # Both collectives issue back-to-back as early as possible (they
            # serialize on the collective cores and dominate the critical
            # path); X waves stream after a8 since Y can't build before the
            # gathers return anyway.
            YENG = ["dve", "act", "dve", "act", "dve"]

            emit_dma_a8(0)
            emit_colsum(0)
            emit_ag_write(0)          # SP throttle: queue hole at cs0
            emit_collective(0)

            emit_dma_a8(1)
            emit_colsum(1)
            emit_ag_write(1)          # SP throttle: queue hole at cs1
            emit_collective(1)

            nc.sync.dma_start(
                xl[:], xlb[:].rearrange("p (t c) -> p t c", c=F))
            emit_dma_x(0)
            emit_dma_x(1)

            emit_xsq_local(["dve", "act"])
            emit_rg(0)
            emit_rinv(0)
            emit_y(0, YENG)
            # sub-cells w'<=k' with y-waves 0-1 (unlocked by gather 0)
            emit_cell(0, 0, start=True, stop=True, drain=True)
            emit_cell(0, 1, start=True, stop=False)
            emit_cell(1, 1, start=False, stop=True, drain=True)
            emit_cell(0, 2, start=True, stop=False)
            emit_cell(1, 2, start=False, stop=False)
            emit_cell(0, 3, start=True, stop=False)
            emit_cell(1, 3, start=False, stop=False)

            emit_rg(1)
            emit_rinv(1)
            emit_y(1, YENG)
            nc.sync.dma_start(out_xsq[:], xsq[:])
            nc.sync.dma_start(out_h[:], degm[:])
            emit_cell(2, 2, start=False, stop=True, drain=True)
            emit_cell(2, 3, start=False, stop=False)
            emit_cell(3, 3, start=False, stop=True, drain=True)
            nc.sync.dma_start(out_q[:], q8[:])

    nc.compile()
    return nc


def _get_nc():
    if "nc" not in _CACHE:
        _CACHE["nc"] = _build_bass()
    return _CACHE["nc"]


def _pack_tiles(arr, rows_per_tile=128):
    """[n*128, c] -> [128, n*c] partition-contiguous (p-major tiles)."""
    n = arr.shape[0] // rows_per_tile
    c = arr.shape[1]
    return np.ascontiguousarray(
        arr.reshape(n, rows_per_tile, c).transpose(1, 0, 2).reshape(
            rows_per_tile, n * c))


def _host_inputs(adj, X):
    import ml_dtypes
    f8 = ml_dtypes.float8_e4m3

    A = (0.5 * (adj + adj.T)).astype(f8)
    Xb = np.asarray(X, dtype=np.float32).astype(f8)
    xb_p = _pack_tiles(Xb)          # [128, T*F]

    in_maps = []
    for c in range(M):
        blk = A[:, c * C:(c + 1) * C]
        parts = []
        for k in range(NCH):
            o = COFF[k] * 128
            w = CHUNKS[k] * 128
            sub = np.ascontiguousarray(blk[:, o:o + w])
            # halve the sub-diagonal at width-2 (256-col) cell granularity:
            # the triangle scheme then needs only ONE drain per block
            # (term2 = 2*sum r*q); exact in fp8 (exponent decrement).  The
            # on-device colsum resum restores the exact column sums.
            rmod = np.arange(N) % C
            for sblk in range(w // 256):
                lo = o + sblk * 256
                mask = (rmod >= lo) & (rmod < lo + 256)
                sub[mask, sblk * 256:(sblk + 1) * 256] = (
                    sub[mask, sblk * 256:(sblk + 1) * 256].astype(np.float32)
                    * 0.5).astype(sub.dtype)
            parts.append(_pack_tiles(sub))
        adjb = np.concatenate(parts, axis=1)
        xlb = _pack_tiles(np.ascontiguousarray(Xb[c * C:(c + 1) * C, :]))
        in_maps.append({"adjb": adjb, "xb": xb_p, "xlb": xlb})
    return in_maps


def kernel(adj: np.ndarray, X: np.ndarray) -> np.ndarray:
    from concourse import bass_utils

    adj = np.asarray(adj, dtype=np.float32)
    X = np.ascontiguousarray(np.asarray(X, dtype=np.float32))
    nc = _get_nc()
    in_maps = _host_inputs(adj, X)

    res = bass_utils.run_bass_kernel_spmd(nc, in_maps, core_ids=list(range(M)))
    results = res.results

    # host-side O(N) reduction (gather/unshard glue)
    deg = results[0]["out_h"].astype(np.float64).T.reshape(-1)
    xsq = np.empty(N, dtype=np.float64)
    for c in range(M):
        xsq[c * C:(c + 1) * C] = results[c]["out_xsq"].astype(
            np.float64).T.reshape(-1)
    w = deg / (deg + EPS)
    rinv = 1.0 / np.sqrt(deg + EPS)
    term1 = float(np.dot(w, xsq))

    q = np.empty(N, dtype=np.float64)
    for c in range(M):
        q[c * C:(c + 1) * C] = results[c]["out_q"].astype(np.float64).T.reshape(-1)
    term2 = 2.0 * float(np.dot(rinv, q)) / YS

    return np.float32(term1 - term2)


if __name__ == "__main__":
    rng = np.random.default_rng(0)
    adj = rng.random((N, N), dtype=np.float32)
    X = rng.standard_normal((N, F), dtype=np.float32)
    print("loss:", kernel(adj, X))
